# revision 16
# baseline (speedup 1.0000x reference)
"""Trainium2 Bass kernel for nn_EssentialMatrixEstimator.

Distribution: data-parallel over the N=3072 rows of Pc across 8 cores
(384 rows each).

Math: the (N*M, 9) epipolar design-matrix Gram collapses to small monomial
Grams. Two phases to match reference f32 conditioning:
  A) C_raw = M1^T W M2 (6x6, raw-coordinate monomials); its row/col 5 hold
     the weighted moments that define the Hartley normalizations T1/T2.
  B) rebuild monomials from *centered* coordinates x^ = s(x - c) (no
     cancellation) and redo the 6x6 Gram -> C2; Mmat (9x9) is then a pure
     index expansion Mmat[3p+q,3r+s] = C2[pair(p,r), pair(q,s)].
W is the bidirectional-top3 (+ >0.01) masked score matrix; exact top-3 with
multiplicity via the hardware Max8 instruction. Column thresholds need all
rows: per-core partials are AllGathered and combined with another Max8.
The 50-step power iterations run as rescaled repeated squaring
(M <- 2*(M@M)); scale/sign drop out of the final normalized eigvectors.
Sign-fix dets are provably +1 and omitted. Validated against reference.
"""

import os

os.environ.setdefault("JAX_PLATFORMS", "axon")

import numpy as np

import concourse.bass as bass
import concourse.bass_isa as bass_isa
import concourse.mybir as mybir
import concourse.bacc as bacc
import concourse.tile as tile

NCORES = 8
N = 3072
SH = N // NCORES          # 384 rows per core
RT = SH // 128            # 3 row tiles per core
CB = N // 128             # 24 column blocks
F32 = mybir.dt.float32
AF = mybir.ActivationFunctionType
OP = mybir.AluOpType
AX = mybir.AxisListType

EPS = 1e-8
SQRT2 = 1.4142135623730951
INV_SQRT3 = 1.0 / 1.7320508075688772
T0 = float(np.nextafter(np.float32(0.01), np.float32(1)))  # x > 0.01 == x >= T0
H, W = 64, 64

# how many of the 24 mask-blocks the DVE handles (rest go to gpsimd)
DVE_BLOCKS = 9

# cpack const layout (tensor [9, 36]): column ranges
C_I9H = 0      # I9 * 0.5          [9, 9]
C_ET69 = 9     # E^T selector      [6, 9]
C_I3 = 18      # I3                [3, 3]
C_V09 = 21     # full(1/3)         [9, 1]
C_V06 = 22     # full(1/sqrt3)     [6, 1]
C_SEL1 = 23    # [I3 | 0]          [3, 6]
C_SEL2 = 29    # [0 | I3]          [3, 6]
C_E5 = 35      # e5 selector       [6, 1]

PAIRS = [(0, 0), (0, 1), (0, 2), (1, 1), (1, 2), (2, 2)]


def _pidx():
    d = {}
    for i, (a, b) in enumerate(PAIRS):
        d[(a, b)] = i
        d[(b, a)] = i
    return d


def host_constants(K):
    """Monomial matrices + packed tail constants (all f32, mirrors reference)."""
    idx = np.arange(H * W, dtype=np.float32)
    pix = np.stack([idx % np.float32(W), np.floor(idx / np.float32(W))], -1)
    K_inv = np.linalg.inv(np.asarray(K, np.float32)).astype(np.float32)
    p1h = np.concatenate([pix[:N], np.ones((N, 1), np.float32)], -1)
    pts = (p1h @ K_inv.T)[:, :2].astype(np.float32)  # same grid both sides
    x, y = pts[:, 0], pts[:, 1]
    M = np.stack([x * x, x * y, x, y * y, y, np.ones_like(x)], -1).astype(np.float32)

    cpack = np.zeros((9, 36), np.float32)
    cpack[:9, C_I9H:C_I9H + 9] = 0.5 * np.eye(9, dtype=np.float32)
    pid = _pidx()
    for a in range(3):
        for b in range(3):
            cpack[pid[(a, b)], C_ET69 + 3 * a + b] = 1.0  # ET69[m, 3a+b]
    cpack[:3, C_I3:C_I3 + 3] = np.eye(3, dtype=np.float32)
    cpack[:9, C_V09] = 1.0 / 3.0
    cpack[:6, C_V06] = INV_SQRT3
    cpack[:3, C_SEL1:C_SEL1 + 3] = np.eye(3, dtype=np.float32)
    cpack[:3, C_SEL2 + 3:C_SEL2 + 6] = np.eye(3, dtype=np.float32)
    cpack[5, C_E5] = 1.0
    return M, cpack


def _tile128(a, ntiles):
    """[ntiles*128, F] -> [128, ntiles*F] with [p, t*F+f] = a[t*128+p, f]."""
    F = a.shape[1]
    return np.ascontiguousarray(
        a.reshape(ntiles, 128, F).transpose(1, 0, 2).reshape(128, ntiles * F)
    )


def _act_copy(nc, out, in_, scale=1.0):
    nc.scalar.activation(out, in_, AF.Copy, scale=scale)


def build_nc():
    """Build the SPMD 8-core Bass program; returns compiled nc."""
    nc = bacc.Bacc("TRN2", target_bir_lowering=False, debug=False,
                   num_devices=NCORES)

    xin = nc.dram_tensor("xin", [128, RT * N], F32, kind="ExternalInput")
    m1s = nc.dram_tensor("m1s", [128, RT * 6], F32, kind="ExternalInput")
    m2t = nc.dram_tensor("m2t", [128, CB * 6], F32, kind="ExternalInput")
    ident = nc.dram_tensor("ident", [128, 128], F32, kind="ExternalInput")
    cpk = nc.dram_tensor("cpack", [9, 36], F32, kind="ExternalInput")
    out_d = nc.dram_tensor("out", [3, 3], F32, kind="ExternalOutput")

    cp_in = nc.dram_tensor("cp_in", [128, CB * 3], F32)
    cp_out = nc.dram_tensor("cp_out", [NCORES * 128, CB * 3], F32,
                            addr_space="Shared")
    cr_in = nc.dram_tensor("cr_in", [6, 6], F32)
    cr_out = nc.dram_tensor("cr_out", [6, 6], F32, addr_space="Shared")
    c2_in = nc.dram_tensor("c2_in", [6, 6], F32)
    c2_out = nc.dram_tensor("c2_out", [6, 6], F32, addr_space="Shared")
    stage = nc.dram_tensor("stage", [64], F32)
    mshuf = nc.dram_tensor("mshuf", [81], F32)

    groups = [list(range(NCORES))]

    with tile.TileContext(nc) as tc:
        with (
            tc.tile_pool(name="persist", bufs=1) as pp,
            tc.tile_pool(name="scratch", bufs=2) as sp,
            tc.tile_pool(name="ps_pt", bufs=2, space="PSUM") as ps,
            tc.tile_pool(name="ps_acc", bufs=2, space="PSUM") as psa,
            tc.tile_pool(name="ps_c", bufs=1, space="PSUM") as psc,
        ):
            # ---------- P0: loads ----------
            X = pp.tile([128, RT * N], F32, tag="X")
            nc.sync.dma_start(X[:], xin[:])
            m1t_s = pp.tile([128, RT * 6], F32, tag="m1")
            nc.sync.dma_start(m1t_s[:], m1s[:])
            m2t_s = pp.tile([128, CB * 6], F32, tag="m2")
            nc.sync.dma_start(m2t_s[:], m2t[:])
            idn = pp.tile([128, 128], F32, tag="idn")
            nc.sync.dma_start(idn[:], ident[:])
            cps = pp.tile([9, 36], F32, tag="cpk")
            nc.sync.dma_start(cps[:], cpk[:])

            def Xt(t):
                return X[:, t * N:(t + 1) * N]

            # ---------- P1: row thresholds ----------
            r8 = pp.tile([128, RT * 8], F32, tag="r8")
            for t in range(RT):
                nc.vector.max(out=r8[:, t * 8:t * 8 + 8], in_=Xt(t))
            trRow = pp.tile([1, SH], F32, tag="trRow")
            for t in range(RT):
                ptr = ps.tile([1, 128], F32, tag="pt")
                nc.tensor.transpose(ptr[:], r8[:, t * 8 + 2:t * 8 + 3], idn[:])
                nc.scalar.activation(trRow[:, t * 128:(t + 1) * 128], ptr[:],
                                     AF.Copy)
            trRow2 = pp.tile([1, SH], F32, tag="trRow2")
            nc.vector.tensor_scalar_max(trRow2[:], trRow[:], T0)
            trB = pp.tile([128, SH], F32, tag="trB")
            nc.gpsimd.partition_broadcast(trB[:], trRow2[:], channels=128)

            # ---------- P2: transposes + column-top3 partials ----------
            XT = pp.tile([128, CB * SH], F32, tag="XT")  # [p=col, (j, r)]
            c8all = pp.tile([128, CB * 8], F32, tag="c8all")
            for j in range(CB):
                for t in range(RT):
                    pt = ps.tile([128, 128], F32, tag="pt")
                    nc.tensor.transpose(
                        pt[:], Xt(t)[:, j * 128:(j + 1) * 128], idn[:])
                    nc.scalar.activation(
                        XT[:, j * SH + t * 128: j * SH + (t + 1) * 128],
                        pt[:], AF.Copy)
                nc.vector.max(out=c8all[:, j * 8:j * 8 + 8],
                              in_=XT[:, j * SH:(j + 1) * SH])
            c3all = pp.tile([128, CB * 3], F32, tag="c3all")
            nc.vector.tensor_copy(
                c3all[:].rearrange("p (j s) -> p j s", s=3),
                c8all[:].rearrange("p (j s) -> p j s", s=8)[:, :, 0:3])
            nc.sync.dma_start(cp_in[:], c3all[:])

            # ---------- collective 1: AllGather column partials ----------
            nc.gpsimd.collective_compute(
                "AllGather", OP.bypass, replica_groups=groups,
                ins=[cp_in[:]], outs=[cp_out[:]])

            gath = pp.tile([128, NCORES * CB * 3], F32, tag="gath")
            nc.sync.dma_start(
                gath[:].rearrange("p (k f) -> p k f", k=NCORES),
                cp_out[:].rearrange("(k p) f -> p k f", p=128))

            # ---------- P3: combine -> exact column thresholds ----------
            cm8 = pp.tile([128, CB * 8], F32, tag="cm8")
            gv = gath[:].rearrange("p (k j s) -> p j k s", k=NCORES, s=3)
            for j in range(CB):
                nc.vector.max(out=cm8[:, j * 8:j * 8 + 8], in_=gv[:, j])

            # ---------- P4: masking (transposed space) ----------
            # thr = max(trB, tc, T0); in place: thr <- [XT >= thr],
            # XT <- XT * thr  (XT becomes the masked W^T)
            thr = pp.tile([128, CB * SH], F32, tag="thr")
            for j in range(CB):
                nc.vector.tensor_scalar_max(
                    thr[:, j * SH:(j + 1) * SH], trB[:],
                    cm8[:, j * 8 + 2:j * 8 + 3])
            half = (CB // 2) * SH
            nc.vector.tensor_tensor(thr[:, :half], XT[:, :half],
                                    thr[:, :half], OP.is_ge)
            nc.vector.tensor_tensor(thr[:, half:], XT[:, half:],
                                    thr[:, half:], OP.is_ge)
            nc.vector.tensor_tensor(XT[:, :half], XT[:, :half],
                                    thr[:, :half], OP.mult)
            nc.vector.tensor_tensor(XT[:, half:], XT[:, half:],
                                    thr[:, half:], OP.mult)

            # ---------- phase-A Gram: C_raw = M1^T W M2 ----------
            def gram(m1_t, m2_t, pc_tile, tagb):
                Bsb = sp.tile([128, RT * 6], F32, tag=tagb)
                for t in range(RT):
                    pb = psa.tile([128, 6], F32, tag="pb")
                    for j in range(CB):
                        nc.tensor.matmul(
                            pb[:],
                            XT[:, j * SH + t * 128: j * SH + (t + 1) * 128],
                            m2_t[:, j * 6:(j + 1) * 6],
                            start=(j == 0), stop=(j == CB - 1))
                    nc.scalar.activation(Bsb[:, t * 6:(t + 1) * 6], pb[:],
                                         AF.Copy)
                for t in range(RT):
                    nc.tensor.matmul(pc_tile[:], m1_t[:, t * 6:(t + 1) * 6],
                                     Bsb[:, t * 6:(t + 1) * 6],
                                     start=(t == 0), stop=(t == RT - 1))

            pc1 = psc.tile([6, 6], F32, tag="pc1")
            gram(m1t_s, m2t_s, pc1, "Bsb1")
            Cp = sp.tile([6, 6], F32, tag="Cp")
            _act_copy(nc, Cp[:], pc1[:])
            nc.sync.dma_start(cr_in[:], Cp[:])

            # ---------- collective 2: AllReduce raw 6x6 Gram ----------
            nc.gpsimd.collective_compute(
                "AllReduce", OP.add, replica_groups=groups,
                ins=[cr_in[:]], outs=[cr_out[:]])

            # ---------- tail part A: Hartley scalars from moments ----------
            sc, nrmB = _hartley(nc, pp, sp, ps, cps, idn, cr_out, stage)

            # ---------- phase-B Gram on centered monomials ----------
            M1n = pp.tile([128, RT * 6], F32, tag="M1n")
            M2n = pp.tile([128, CB * 6], F32, tag="M2n")

            def build_norm(src, dst, nt, sB, cxB, cyB, tagn):
                sv = src[:].rearrange("p (t d) -> p d t", d=6)
                dv = dst[:].rearrange("p (t d) -> p d t", d=6)
                xh = sp.tile([128, nt], F32, tag=f"xh{tagn}")
                nc.vector.tensor_scalar(xh[:], sv[:, 2], cxB, sB,
                                        OP.subtract, OP.mult)
                yh = sp.tile([128, nt], F32, tag=f"yh{tagn}")
                nc.vector.tensor_scalar(yh[:], sv[:, 4], cyB, sB,
                                        OP.subtract, OP.mult)
                nc.vector.tensor_tensor(dv[:, 0], xh[:], xh[:], OP.mult)
                nc.vector.tensor_tensor(dv[:, 1], xh[:], yh[:], OP.mult)
                nc.vector.tensor_copy(dv[:, 2], xh[:])
                nc.vector.tensor_tensor(dv[:, 3], yh[:], yh[:], OP.mult)
                nc.vector.tensor_copy(dv[:, 4], yh[:])
                nc.vector.memset(dv[:, 5], 1.0)

            # nrmB cols: [s1, c1x, c1y, s2, c2x, c2y]
            build_norm(m1t_s, M1n, RT, nrmB[:, 0:1], nrmB[:, 1:2],
                       nrmB[:, 2:3], "1")
            build_norm(m2t_s, M2n, CB, nrmB[:, 3:4], nrmB[:, 4:5],
                       nrmB[:, 5:6], "2")

            pc2 = psc.tile([6, 6], F32, tag="pc2")
            gram(M1n, M2n, pc2, "Bsb2")
            C2p = sp.tile([6, 6], F32, tag="C2p")
            _act_copy(nc, C2p[:], pc2[:])
            nc.sync.dma_start(c2_in[:], C2p[:])

            # ---------- collective 3: AllReduce normalized 6x6 Gram ------
            nc.gpsimd.collective_compute(
                "AllReduce", OP.add, replica_groups=groups,
                ins=[c2_in[:]], outs=[c2_out[:]])

            # ---------- tail part B ----------
            _solve(nc, pp, sp, ps, cps, idn, sc, c2_out, stage, mshuf, out_d)

    nc.compile()
    return nc


def _transpose(nc, ps, sp, in_sb, n, idn, tag):
    """PE-transpose square [n, n] SBUF -> new SBUF tile."""
    pt = ps.tile([n, n], F32, tag="tps")
    nc.tensor.transpose(pt[:], in_sb, idn[:n, :n])
    ot = sp.tile([n, n], F32, tag=f"ot_{tag}")
    _act_copy(nc, ot[:], pt[:])
    return ot


def _pow50(nc, ps, sp, m_sb, n, tag):
    """Direction of M^50 v via rescaled squarings M <- 2*(M@M);
    M50 = 2*((2*(M32@M16)) @ M2). All operands symmetric."""
    powers = {}
    cur = m_sb
    for i in range(1, 6):  # M2, M4, M8, M16, M32
        pm = ps.tile([n, n], F32, tag="tps")
        nc.tensor.matmul(pm[:], cur, cur, start=True, stop=True)
        nxt = sp.tile([n, n], F32, tag=f"pws_{tag}_{i}")
        _act_copy(nc, nxt[:], pm[:], scale=2.0)
        powers[2 ** i] = nxt
        cur = nxt[:]
    pm = ps.tile([n, n], F32, tag="tps")
    nc.tensor.matmul(pm[:], powers[32][:], powers[16][:], start=True, stop=True)
    m48 = sp.tile([n, n], F32, tag=f"pws_{tag}_48")
    _act_copy(nc, m48[:], pm[:], scale=2.0)
    pm = ps.tile([n, n], F32, tag="tps")
    nc.tensor.matmul(pm[:], m48[:], powers[2][:], start=True, stop=True)
    m50 = sp.tile([n, n], F32, tag=f"pws_{tag}_50")
    _act_copy(nc, m50[:], pm[:], scale=2.0)
    return m50


def _hartley(nc, pp, sp, ps, cps, idn, cr_out, stage):
    """Moments -> Hartley scalars on partition 0; stage T1/T2 row-major;
    return (sc scratch tile, nrmB [128, 6] = bcast [s1,c1x,c1y,s2,c2x,c2y])."""
    e5 = cps[0:6, C_E5:C_E5 + 1]

    Cr = sp.tile([6, 6], F32, tag="Cr")
    nc.sync.dma_start(Cr[:], cr_out[:])
    CrT = _transpose(nc, ps, sp, Cr[:], 6, idn, "crt")

    sc = pp.tile([128, 96], F32, tag="tailsc")

    def scv(a, b):
        return sc[0:1, a:b]

    mo_ps = ps.tile([1, 6], F32, tag="tps")
    nc.tensor.matmul(mo_ps[:], e5, CrT[:], start=True, stop=True)
    _act_copy(nc, scv(0, 6), mo_ps[:])              # side1 moments
    mo_ps2 = ps.tile([1, 6], F32, tag="tps")
    nc.tensor.matmul(mo_ps2[:], e5, Cr[:], start=True, stop=True)
    _act_copy(nc, scv(6, 12), mo_ps2[:])            # side2 moments

    def pair(k):  # element k of each side: free idxs (k, k+6)
        return sc[0:1, 0:12].rearrange("p (g d) -> p d g", g=2)[:, k, :]

    Sxx, Sx, Syy, Sy, Sw = pair(0), pair(2), pair(3), pair(4), pair(5)
    ws = scv(12, 14); nc.vector.tensor_scalar_add(ws, Sw, EPS)
    rws = scv(14, 16); nc.vector.reciprocal(rws, ws)
    cx = scv(16, 18); nc.vector.tensor_tensor(cx, Sx, rws, OP.mult)
    cy = scv(18, 20); nc.vector.tensor_tensor(cy, Sy, rws, OP.mult)
    t_a = scv(20, 22); nc.vector.tensor_tensor(t_a, cx, Sx, OP.mult)
    t_b = scv(22, 24); nc.vector.tensor_tensor(t_b, cy, Sy, OP.mult)
    cdS = scv(24, 26); nc.vector.tensor_tensor(cdS, t_a, t_b, OP.add)
    u_a = scv(26, 28); nc.vector.tensor_tensor(u_a, cx, cx, OP.mult)
    u_b = scv(28, 30); nc.vector.tensor_tensor(u_b, cy, cy, OP.mult)
    c2_ = scv(30, 32); nc.vector.tensor_tensor(c2_, u_a, u_b, OP.add)
    sq_ = scv(32, 34); nc.vector.tensor_tensor(sq_, Sxx, Syy, OP.add)
    n2c = scv(34, 36); nc.vector.tensor_scalar_mul(n2c, cdS, -2.0)
    c2w = scv(36, 38); nc.vector.tensor_tensor(c2w, c2_, Sw, OP.mult)
    m_ = scv(38, 40); nc.vector.tensor_tensor(m_, sq_, n2c, OP.add)
    m2_ = scv(40, 42); nc.vector.tensor_tensor(m2_, m_, c2w, OP.add)
    md2 = scv(42, 44); nc.vector.tensor_tensor(md2, m2_, rws, OP.mult)
    md2e = scv(44, 46); nc.vector.tensor_scalar_add(md2e, md2, EPS)
    md = scv(46, 48); nc.scalar.activation(md, md2e, AF.Sqrt)
    mde = scv(48, 50); nc.vector.tensor_scalar_add(mde, md, EPS)
    rmd = scv(50, 52); nc.vector.reciprocal(rmd, mde)
    s_ = scv(52, 54); nc.vector.tensor_scalar_mul(s_, rmd, SQRT2)
    scx = scv(54, 56); nc.vector.tensor_tensor(scx, s_, cx, OP.mult)
    scy = scv(56, 58); nc.vector.tensor_tensor(scy, s_, cy, OP.mult)
    nscx = scv(58, 60); nc.vector.tensor_scalar_mul(nscx, scx, -1.0)
    nscy = scv(60, 62); nc.vector.tensor_scalar_mul(nscy, scy, -1.0)

    # T row-major 9-vectors: t1v at 64:73, t2v at 73:82
    nc.vector.memset(scv(64, 82), 0.0)
    tv = sc[0:1, 64:82]
    tv9 = tv.rearrange("p (v f) -> p v f", v=2)  # [1, 2(side), 9]
    nc.vector.tensor_copy(tv9[:, :, 0:1], s_.unsqueeze(2))
    nc.vector.tensor_copy(tv9[:, :, 4:5], s_.unsqueeze(2))
    nc.vector.tensor_copy(
        tv9[:, :, 2:8].rearrange("p v (c d) -> p v c d", c=2)[:, :, :, 0:1],
        sc[0:1, 58:62].rearrange("p (c v) -> p v c", c=2).unsqueeze(3))
    nc.vector.memset(tv9[:, :, 8:9], 1.0)
    nc.gpsimd.dma_start(stage[0:18], tv)

    # normalization scalar vector [s1, c1x, c1y, s2, c2x, c2y] -> bcast
    nv = scv(84, 90)
    nc.vector.tensor_copy(sc[0:1, 84:85], sc[0:1, 52:53])   # s1
    nc.vector.tensor_copy(
        sc[0:1, 85:87],
        sc[0:1, 16:20].rearrange("p (d g) -> p d g", d=2)[:, :, 0])  # c1x c1y
    nc.vector.tensor_copy(sc[0:1, 87:88], sc[0:1, 53:54])   # s2
    nc.vector.tensor_copy(
        sc[0:1, 88:90],
        sc[0:1, 16:20].rearrange("p (d g) -> p d g", d=2)[:, :, 1])  # c2x c2y
    nrmB = pp.tile([128, 6], F32, tag="nrmB")
    nc.gpsimd.partition_broadcast(nrmB[:], nv, channels=128)
    return sc, nrmB


def _solve(nc, pp, sp, ps, cps, idn, sc, c2_out, stage, mshuf, out_d):
    """Mmat expansion, power chains, projection, output."""
    i9h = cps[0:9, C_I9H:C_I9H + 9]
    et69 = cps[0:6, C_ET69:C_ET69 + 9]
    i3c = cps[0:3, C_I3:C_I3 + 3]
    v09 = cps[0:9, C_V09:C_V09 + 1]
    v06 = cps[0:6, C_V06:C_V06 + 1]
    sel1 = cps[0:3, C_SEL1:C_SEL1 + 6]
    sel2 = cps[0:3, C_SEL2:C_SEL2 + 6]

    C2r = sp.tile([6, 6], F32, tag="C2r")
    nc.sync.dma_start(C2r[:], c2_out[:])
    C2rT = _transpose(nc, ps, sp, C2r[:], 6, idn, "c2rt")

    # G2 = E C2 E^T : G2[3a+b, 3c+d] = C2[pair(a,b), pair(c,d)]
    z_ps = ps.tile([6, 9], F32, tag="tps")
    nc.tensor.matmul(z_ps[:], C2rT[:], et69, start=True, stop=True)  # C2 E^T
    Zs = sp.tile([6, 9], F32, tag="Zs")
    _act_copy(nc, Zs[:], z_ps[:])
    g_ps = ps.tile([9, 9], F32, tag="tps")
    nc.tensor.matmul(g_ps[:], et69, Zs[:], start=True, stop=True)    # E @ Z
    G2 = sp.tile([9, 9], F32, tag="G2")
    _act_copy(nc, G2[:], g_ps[:])

    # Mmat[3p+q, 3r+s] = G2[3p+r, 3q+s]: bounce via DRAM, 9 row reads
    nc.gpsimd.dma_start(mshuf[:], G2[:])
    Mmat = sp.tile([9, 9], F32, tag="Mmat")
    for p in range(3):
        for q in range(3):
            nc.gpsimd.dma_start(
                Mmat[3 * p + q:3 * p + q + 1, :],
                mshuf[:].rearrange("(p q1 r s) -> p q1 r s",
                                   p=3, q1=3, r=3)
                .transpose([0, 2, 1, 3])[p, q].unsqueeze(0))

    # shifted scaled 9x9: Msp = Mmat/(2 lam) - I/2 (sign irrelevant, even pow)
    dg = sp.tile([9, 9], F32, tag="dg")
    nc.vector.tensor_tensor(dg[:], Mmat[:], i9h, OP.mult)  # diag/2
    lam2 = sp.tile([9, 1], F32, tag="lam2")
    nc.vector.tensor_reduce(lam2[:], dg[:], AX.X, OP.add)
    lam2r = sp.tile([9, 1], F32, tag="lam2r")
    nc.gpsimd.partition_all_reduce(lam2r[:], lam2[:], channels=9,
                                   reduce_op=bass_isa.ReduceOp.add)
    lam4 = sp.tile([9, 1], F32, tag="lam4")
    nc.vector.tensor_scalar_mul(lam4[:], lam2r[:], 4.0)  # = 2*lam
    inv2l = sp.tile([9, 1], F32, tag="inv2l")
    nc.vector.reciprocal(inv2l[:], lam4[:])
    Msp = sp.tile([9, 9], F32, tag="Msp")
    nc.vector.scalar_tensor_tensor(Msp[:], Mmat[:], inv2l[:], i9h,
                                   OP.mult, OP.subtract)
    M50 = _pow50(nc, ps, sp, Msp[:], 9, "m9")

    w9ps = ps.tile([1, 9], F32, tag="tps")
    nc.tensor.matmul(w9ps[:], v09, M50[:], start=True, stop=True)
    w9 = sp.tile([1, 9], F32, tag="w9")
    _act_copy(nc, w9[:], w9ps[:])
    w9sq = sp.tile([1, 9], F32, tag="w9sq")
    nc.vector.tensor_tensor(w9sq[:], w9[:], w9[:], OP.mult)
    nn9 = sp.tile([1, 1], F32, tag="nn9")
    nc.vector.tensor_reduce(nn9[:], w9sq[:], AX.X, OP.add)
    sr9 = sp.tile([1, 1], F32, tag="sr9")
    nc.scalar.activation(sr9[:], nn9[:], AF.Sqrt)
    rs9 = sp.tile([1, 1], F32, tag="rs9")
    nc.vector.reciprocal(rs9[:], sr9[:])
    v9 = sp.tile([1, 9], F32, tag="v9")
    nc.vector.tensor_tensor(v9[:], w9[:], rs9[:].to_broadcast([1, 9]), OP.mult)
    nc.gpsimd.dma_start(stage[24:33], v9[:])

    # E = T2^T E_raw T1 (and E^T)
    T1m = sp.tile([3, 3], F32, tag="T1m")
    nc.gpsimd.dma_start(T1m[:], stage[0:9].rearrange("(i j) -> i j", j=3))
    T2m = sp.tile([3, 3], F32, tag="T2m")
    nc.gpsimd.dma_start(T2m[:], stage[9:18].rearrange("(i j) -> i j", j=3))
    Eraw = sp.tile([3, 3], F32, tag="Eraw")
    nc.gpsimd.dma_start(Eraw[:], stage[24:33].rearrange("(i j) -> i j", j=3))

    a1ps = ps.tile([3, 3], F32, tag="tps")
    nc.tensor.matmul(a1ps[:], T2m[:], Eraw[:], start=True, stop=True)
    A1 = sp.tile([3, 3], F32, tag="A1")
    _act_copy(nc, A1[:], a1ps[:])
    A1T = _transpose(nc, ps, sp, A1[:], 3, idn, "a1t")
    etps = ps.tile([3, 3], F32, tag="tps")
    nc.tensor.matmul(etps[:], T1m[:], A1T[:], start=True, stop=True)
    ETs = sp.tile([3, 3], F32, tag="ETs")
    _act_copy(nc, ETs[:], etps[:])
    Es = _transpose(nc, ps, sp, ETs[:], 3, idn, "es")

    # B = E^T E ; blockdiag 6x6 chain for v1 (max) and v3 (min)
    bps = ps.tile([3, 3], F32, tag="tps")
    nc.tensor.matmul(bps[:], Es[:], Es[:], start=True, stop=True)
    Bm = sp.tile([3, 3], F32, tag="Bm")
    _act_copy(nc, Bm[:], bps[:])
    dg3 = sp.tile([3, 3], F32, tag="dg3")
    nc.vector.tensor_tensor(dg3[:], Bm[:], i3c, OP.mult)
    lb = sp.tile([3, 1], F32, tag="lb")
    nc.vector.tensor_reduce(lb[:], dg3[:], AX.X, OP.add)
    lbr = sp.tile([3, 1], F32, tag="lbr")
    nc.gpsimd.partition_all_reduce(lbr[:], lb[:], channels=3,
                                   reduce_op=bass_isa.ReduceOp.add)
    invlb = sp.tile([3, 1], F32, tag="invlb")
    nc.vector.reciprocal(invlb[:], lbr[:])
    Bs3 = sp.tile([3, 3], F32, tag="Bs3")
    nc.vector.tensor_scalar_mul(Bs3[:], Bm[:], invlb[:])
    IB = sp.tile([3, 3], F32, tag="IB")
    nc.vector.tensor_tensor(IB[:], i3c, Bs3[:], OP.subtract)
    bdps = ps.tile([6, 6], F32, tag="tps")
    nc.tensor.matmul(bdps[:, 0:3], sel1, Bs3[:], start=True, stop=True)
    nc.tensor.matmul(bdps[:, 3:6], sel2, IB[:], start=True, stop=True)
    BD = sp.tile([6, 6], F32, tag="BD")
    _act_copy(nc, BD[:], bdps[:])
    BD50 = _pow50(nc, ps, sp, BD[:], 6, "m6")

    w6ps = ps.tile([1, 6], F32, tag="tps")
    nc.tensor.matmul(w6ps[:], v06, BD50[:], start=True, stop=True)
    w6 = sp.tile([1, 6], F32, tag="w6")
    _act_copy(nc, w6[:], w6ps[:])
    w6sq = sp.tile([1, 6], F32, tag="w6sq")
    nc.vector.tensor_tensor(w6sq[:], w6[:], w6[:], OP.mult)
    nn6 = sp.tile([1, 2], F32, tag="nn6")
    nc.vector.tensor_reduce(nn6[:].unsqueeze(2),
                            w6sq[:].rearrange("p (g d) -> p g d", g=2), AX.X,
                            OP.add)
    sr6 = sp.tile([1, 2], F32, tag="sr6")
    nc.scalar.activation(sr6[:], nn6[:], AF.Sqrt)
    rs6 = sp.tile([1, 2], F32, tag="rs6")
    nc.vector.reciprocal(rs6[:], sr6[:])
    vv = sp.tile([1, 6], F32, tag="vv")
    nc.vector.tensor_tensor(
        vv[:].rearrange("p (g d) -> p g d", g=2),
        w6[:].rearrange("p (g d) -> p g d", g=2),
        rs6[:].unsqueeze(2).to_broadcast([1, 2, 3]), OP.mult)

    # v2 = cross(v3, v1), normalized with EPS (as reference)
    aa = sp.tile([1, 6], F32, tag="aa")
    nc.vector.tensor_copy(
        aa[:].rearrange("p (r d) -> p r d", r=2),
        vv[:, 3:6].unsqueeze(1).to_broadcast([1, 2, 3]))
    bb = sp.tile([1, 6], F32, tag="bb")
    nc.vector.tensor_copy(
        bb[:].rearrange("p (r d) -> p r d", r=2),
        vv[:, 0:3].unsqueeze(1).to_broadcast([1, 2, 3]))
    cr1 = sp.tile([1, 3], F32, tag="cr1")
    nc.vector.tensor_tensor(cr1[:], aa[:, 1:4], bb[:, 2:5], OP.mult)
    cr2 = sp.tile([1, 3], F32, tag="cr2")
    nc.vector.tensor_tensor(cr2[:], aa[:, 2:5], bb[:, 1:4], OP.mult)
    v2r = sp.tile([1, 3], F32, tag="v2r")
    nc.vector.tensor_tensor(v2r[:], cr1[:], cr2[:], OP.subtract)
    v2sq = sp.tile([1, 3], F32, tag="v2sq")
    nc.vector.tensor_tensor(v2sq[:], v2r[:], v2r[:], OP.mult)
    nn2 = sp.tile([1, 1], F32, tag="nn2")
    nc.vector.tensor_reduce(nn2[:], v2sq[:], AX.X, OP.add)
    sr2 = sp.tile([1, 1], F32, tag="sr2")
    nc.scalar.activation(sr2[:], nn2[:], AF.Sqrt)
    sr2e = sp.tile([1, 1], F32, tag="sr2e")
    nc.vector.tensor_scalar_add(sr2e[:], sr2[:], EPS)
    rs2 = sp.tile([1, 1], F32, tag="rs2")
    nc.vector.reciprocal(rs2[:], sr2e[:])
    v2 = sp.tile([1, 3], F32, tag="v2")
    nc.vector.tensor_tensor(v2[:], v2r[:], rs2[:].to_broadcast([1, 3]), OP.mult)

    # stage v1, v2; Ev rows; final assembly
    nc.gpsimd.dma_start(stage[33:36], vv[:, 0:3])
    nc.gpsimd.dma_start(stage[36:39], v2[:])
    Vc = sp.tile([3, 2], F32, tag="Vc")
    nc.gpsimd.dma_start(Vc[:], stage[33:39].rearrange("(i k) -> k i", k=3))
    Vr = sp.tile([2, 3], F32, tag="Vr")
    nc.gpsimd.dma_start(Vr[:], stage[33:39].rearrange("(i k) -> i k", k=3))
    evps = ps.tile([2, 3], F32, tag="tps")
    nc.tensor.matmul(evps[:], Vc[:], ETs[:], start=True, stop=True)
    Evr = sp.tile([2, 3], F32, tag="Evr")
    _act_copy(nc, Evr[:], evps[:])
    evsq = sp.tile([2, 3], F32, tag="evsq")
    nc.vector.tensor_tensor(evsq[:], Evr[:], Evr[:], OP.mult)
    ss2 = sp.tile([2, 1], F32, tag="ss2")
    nc.vector.tensor_reduce(ss2[:], evsq[:], AX.X, OP.add)
    sv = sp.tile([2, 1], F32, tag="sv")
    nc.scalar.activation(sv[:], ss2[:], AF.Sqrt)
    ssum = sp.tile([2, 1], F32, tag="ssum")
    nc.gpsimd.partition_all_reduce(ssum[:], sv[:], channels=2,
                                   reduce_op=bass_isa.ReduceOp.add)
    savg = sp.tile([2, 1], F32, tag="savg")
    nc.vector.tensor_scalar_mul(savg[:], ssum[:], 0.5)
    sve = sp.tile([2, 1], F32, tag="sve")
    nc.vector.tensor_scalar_add(sve[:], sv[:], EPS)
    rsv = sp.tile([2, 1], F32, tag="rsv")
    nc.vector.reciprocal(rsv[:], sve[:])
    f2 = sp.tile([2, 1], F32, tag="f2")
    nc.vector.tensor_tensor(f2[:], rsv[:], savg[:], OP.mult)
    U2 = sp.tile([2, 3], F32, tag="U2")
    nc.vector.tensor_scalar_mul(U2[:], Evr[:], f2[:])
    ops_ = ps.tile([3, 3], F32, tag="tps")
    nc.tensor.matmul(ops_[:], U2[:], Vr[:], start=True, stop=True)
    outs = sp.tile([3, 3], F32, tag="outs")
    _act_copy(nc, outs[:], ops_[:])
    nc.sync.dma_start(out_d[:], outs[:])


def make_in_maps(P, K):
    """Host-side shard + constant prep: list of 8 input dicts."""
    P = np.asarray(P, np.float32)
    K = np.asarray(K, np.float32)
    Pc = np.ascontiguousarray(P[:N, :N])
    M, cpack = host_constants(K)
    m2t = _tile128(M, CB)
    ident = np.eye(128, dtype=np.float32)
    in_maps = []
    for k in range(NCORES):
        sh = Pc[k * SH:(k + 1) * SH]
        in_maps.append({
            "xin": _tile128(sh, RT),
            "m1s": _tile128(M[k * SH:(k + 1) * SH], RT),
            "m2t": m2t,
            "ident": ident,
            "cpack": cpack,
        })
    return in_maps


_NC_CACHE = {}


def kernel(P, K):
    from concourse.bass_utils import run_bass_kernel_spmd
    if "nc" not in _NC_CACHE:
        _NC_CACHE["nc"] = build_nc()
    nc = _NC_CACHE["nc"]
    in_maps = make_in_maps(P, K)
    res = run_bass_kernel_spmd(nc, in_maps, core_ids=list(range(NCORES)))
    return np.asarray(res.results[0]["out"], np.float32)
